# revision 11
# baseline (speedup 1.0000x reference)
"""Trainium2 Bass kernel for nn_MoELayer (top-2 MoE, E=8 experts).

Strategy (F-sliced tensor parallel across 8 NeuronCores):
  - Host computes the (tiny) gate matmul + top-2 + softmax and groups the
    8192 token-expert pairs by expert.
  - Every core holds a distinct 512-wide slice of the F dimension of ALL 8
    experts (W1[:, :, fs], W2[:, fs, :]) and processes ALL routed pairs on
    that slice -> perfectly balanced 8192 pairs/core (no max-expert padding),
    vs expert-parallel where the busiest expert (~1070 tokens) sets the pace.
  - Each core computes partial outputs  silu(tok @ W1[e][:, fs]) @ W2[e][fs, :]
    in bf16 with fp32 PSUM accumulation; host sums the 8 partials and
    scatter-adds with the gate weights.

Startup optimizations (the baseline lost ~20us before its first matmul):
  - ~36 dummy 128-wide matmuls on a memset-zero tile issued immediately
    (no DMA dependency) warm the PE HAM clock gate (1.2 -> 2.4 GHz takes
    ~3.4us of sustained PE activity) while the first real data streams in.
  - The first real matmul only needs a 128-token micro-chunk (0.25 MB) plus
    one [P, KD, 128] W1 slice (0.25 MB) instead of 2.6 MB.
  - Weights/tokens are streamed with a 2-expert DMA lookahead; per-expert
    working set is ~4.2 MB so the queue never runs ahead of SBUF space.
  - Input loads ride the sync engine's HW DGE; per-chunk batched output
    DMAs ride the activation engine's HW DGE (separate queue).
"""

import math
import sys

sys.path.insert(0, "/opt/trn_rl_repo")

import ml_dtypes
import numpy as np

B, T, D, F, E = 2, 2048, 1024, 4096, 8
N = B * T          # 4096 tokens
NPAIRS = 2 * N     # 8192 routed token-expert pairs
P = 128
KD = D // P        # 8 contraction blocks for stage 1
DM = D // P        # 8 output-row blocks for stage 2
M = 8              # cores
FS = F // M        # 512 F-columns per core
FJ = FS // P       # 4 f-tiles per core (stage-1 output / stage-2 contraction)

NWARM = 66         # PE warm-up matmuls: bridge preamble->first data (~12.5us)

bf16 = ml_dtypes.bfloat16

_nc_cache: dict[tuple, object] = {}
LAST_RESULTS = None  # BassKernelResults from the most recent run (for test.py)
TRACE = False


def _chunks_for(n: int, first_small: bool, last_small: bool = False) -> list[int]:
    """Exact cover of n with chunks <=512 (PSUM bank limit), ascending.
    first_small puts a 128-token micro-chunk first so the critical first
    DMA (and first matmul) is as early as possible; last_small puts a
    128-token chunk last so the final output DMA is small."""
    if n <= 0:
        return []
    head, tail = [], []
    if first_small and n > 192:
        head.append(128)
        n -= 128
    if last_small and n > 192:
        tail.append(128)
        n -= 128
    k = math.ceil(n / 512)
    base = n // k
    rem = n - base * k
    mid = [base + (1 if i >= k - rem else 0) for i in range(k)]
    if last_small:
        mid = mid[::-1]
    return head + mid + tail


def _chunk_lists(counts) -> list[list[int]]:
    nexp = len(counts)
    return [
        _chunks_for(n, first_small=(i == 0), last_small=(i == nexp - 1))
        for i, n in enumerate(counts)
    ]


def _build(counts: tuple[int, ...]):
    import concourse.mybir as mybir
    import concourse.tile as tile
    from concourse import bacc

    dt = mybir.dt

    nc = bacc.Bacc(None, target_bir_lowering=False)

    nexp = len(counts)
    chunk_lists = _chunk_lists(counts)

    # ---- DRAM tensors ----
    tok_dram = [
        [
            nc.dram_tensor(f"tok{i}_{c}", [P, KD, cn], dt.bfloat16, kind="ExternalInput")
            for c, cn in enumerate(chunks)
        ]
        for i, chunks in enumerate(chunk_lists)
    ]
    # expert 0: first token chunk and first W1 slice fused into one tensor
    # (one DMA + one semaphore on the critical path to the first matmul),
    # remaining W1 slices fine-grained
    c00 = chunk_lists[0][0]
    first0_dram = nc.dram_tensor(
        "first0", [P, KD, c00 + P], dt.bfloat16, kind="ExternalInput"
    )
    w1_dram0 = [
        nc.dram_tensor(f"w1_0_{j}", [P, KD, P], dt.bfloat16, kind="ExternalInput")
        for j in range(1, FJ)
    ]
    w1_dram = [
        nc.dram_tensor(f"w1_{i}", [P, FJ, KD, P], dt.bfloat16, kind="ExternalInput")
        for i in range(1, nexp)
    ]
    w2_dram = [
        nc.dram_tensor(f"w2_{i}", [P, FJ, D], dt.bfloat16, kind="ExternalInput")
        for i in range(nexp)
    ]
    # partial outputs, bf16, one tensor per chunk so each DMA writes a
    # fully contiguous 5.5KB line per partition: [p, dm, c] with d = dm*128 + p
    out_dram = [
        [
            nc.dram_tensor(
                f"out{i}_{c}", [P, DM, cn], dt.bfloat16, kind="ExternalOutput"
            )
            for c, cn in enumerate(chunks)
        ]
        for i, chunks in enumerate(chunk_lists)
    ]

    with tile.TileContext(nc) as tc:
        with (
            tc.tile_pool(name="z", bufs=1) as zpool,
            tc.tile_pool(name="w", bufs=3) as wpool,
            tc.tile_pool(name="tok", bufs=10) as tokpool,
            tc.tile_pool(name="act", bufs=3) as apool,
            tc.tile_pool(name="ob", bufs=3) as opool,
            tc.tile_pool(name="ps1", bufs=4, space="PSUM") as ps1pool,
            tc.tile_pool(name="ps2", bufs=4, space="PSUM") as ps2pool,
        ):
            # ---- PE warm-up: no DMA dependency, starts right after preamble ----
            zeros = zpool.tile([P, 2 * P], dt.bfloat16, tag="zeros")
            nc.vector.memset(zeros[:], 0.0)
            for k in range(NWARM):
                wps = ps1pool.tile([P, P], dt.float32, tag="ps1")
                nc.tensor.matmul(
                    wps[:], zeros[:, 0:P], zeros[:, P : 2 * P], start=True, stop=True
                )

            tok_sb = [[None] * len(ch) for ch in chunk_lists]
            w1_sb: list = [None] * nexp
            w2_sb: list = [None] * nexp

            def emit_loads(i: int):
                chunks = chunk_lists[i]
                if i == 0:
                    # critical path: fused micro token chunk + first W1 slice
                    first0_sb = tokpool.tile(
                        [P, KD, chunks[0] + P], dt.bfloat16, tag="tok",
                        name="first0",
                    )
                    nc.sync.dma_start(first0_sb[:], first0_dram[:])
                    tok_sb[0][0] = first0_sb
                    w1_sb[0] = [first0_sb] + [
                        wpool.tile(
                            [P, KD, P], dt.bfloat16, tag=f"w1s{j}", name=f"w1s{j}"
                        )
                        for j in range(1, FJ)
                    ]
                    for j in range(1, FJ):
                        nc.sync.dma_start(w1_sb[0][j][:], w1_dram0[j - 1][:])
                    for c in range(1, len(chunks)):
                        tok_sb[0][c] = tokpool.tile(
                            [P, KD, chunks[c]], dt.bfloat16, tag="tok",
                            name=f"tok0_{c}",
                        )
                        nc.sync.dma_start(tok_sb[0][c][:], tok_dram[0][c][:])
                else:
                    for c, cn in enumerate(chunks):
                        tok_sb[i][c] = tokpool.tile(
                            [P, KD, cn], dt.bfloat16, tag="tok",
                            name=f"tok{i}_{c}",
                        )
                        nc.sync.dma_start(tok_sb[i][c][:], tok_dram[i][c][:])
                    w1_sb[i] = wpool.tile(
                        [P, FJ, KD, P], dt.bfloat16, tag="w1", name=f"w1_{i}"
                    )
                    nc.sync.dma_start(w1_sb[i][:], w1_dram[i - 1][:])
                w2_sb[i] = wpool.tile(
                    [P, FJ, D], dt.bfloat16, tag="w2", name=f"w2_{i}"
                )
                nc.sync.dma_start(w2_sb[i][:], w2_dram[i][:])

            emit_loads(0)
            if nexp > 1:
                emit_loads(1)

            outno = 0
            for i in range(nexp):
                if i + 2 < nexp:
                    emit_loads(i + 2)
                chunks = chunk_lists[i]
                # ---- stage 1: actT[f_local, c] = silu(W1s^T @ tokT) ----
                act_sbs = []
                for c, cn in enumerate(chunks):
                    act_sb = apool.tile([P, FJ, cn], dt.bfloat16, tag="act")
                    act_sbs.append(act_sb)
                    c0w = chunk_lists[0][0]
                    for j in range(FJ):
                        ps1 = ps1pool.tile([P, cn], dt.float32, tag="ps1")
                        for dk in range(KD):
                            if i == 0 and j == 0:
                                lhs = w1_sb[0][0][:, dk, c0w : c0w + P]
                            elif i == 0:
                                lhs = w1_sb[0][j][:, dk, :]
                            else:
                                lhs = w1_sb[i][:, j, dk, :]
                            nc.tensor.matmul(
                                ps1[:],
                                lhs,
                                tok_sb[i][c][:, dk, : chunks[0]]
                                if (i == 0 and c == 0)
                                else tok_sb[i][c][:, dk, :],
                                start=(dk == 0),
                                stop=(dk == KD - 1),
                            )
                        nc.scalar.activation(
                            act_sb[:, j, :],
                            ps1[:],
                            mybir.ActivationFunctionType.Silu,
                        )
                # ---- stage 2: outT[d, c] = W2s^T @ actT (partial over f) ----
                for c, cn in enumerate(chunks):
                    ob = opool.tile([P, DM, cn], dt.bfloat16, tag="ob")
                    for dm in range(DM):
                        ps2 = ps2pool.tile([P, cn], dt.float32, tag="ps2")
                        for fk in range(FJ):
                            nc.tensor.matmul(
                                ps2[:],
                                w2_sb[i][:, fk, dm * P : (dm + 1) * P],
                                act_sbs[c][:, fk, :],
                                start=(fk == 0),
                                stop=(fk == FJ - 1),
                            )
                        nc.vector.tensor_copy(ob[:, dm, :], ps2[:])
                    if i == nexp - 1 and c >= len(chunks) - 2:
                        # tail chunks: split the DMA across both HWDGE queues
                        # so the post-compute drain is halved
                        nc.scalar.dma_start(
                            out_dram[i][c][:, : DM // 2, :], ob[:, : DM // 2, :]
                        )
                        nc.sync.dma_start(
                            out_dram[i][c][:, DM // 2 :, :], ob[:, DM // 2 :, :]
                        )
                    else:
                        # alternate output DMAs across the two HWDGE queues
                        eng = nc.scalar if outno % 2 == 0 else nc.sync
                        eng.dma_start(out_dram[i][c][:], ob[:])
                    outno += 1

    nc.compile()
    return nc


def _get_nc(counts: tuple[int, ...]):
    if counts not in _nc_cache:
        _nc_cache[counts] = _build(counts)
    return _nc_cache[counts]


def kernel(**inputs) -> np.ndarray:
    global LAST_RESULTS
    x = np.asarray(inputs["x"], dtype=np.float32)
    Wg = np.asarray(inputs["Wg"], dtype=np.float32)
    W1 = np.asarray(inputs["W1"], dtype=np.float32)
    W2 = np.asarray(inputs["W2"], dtype=np.float32)

    h = np.ascontiguousarray(x.reshape(N, D))

    # ---- host gate: top-2 + softmax (0.05% of total FLOPs) ----
    logits = h @ Wg.T  # [N, E] f32
    idx2 = np.argpartition(-logits, 1, axis=1)[:, :2]
    lsel = np.take_along_axis(logits, idx2, axis=1)
    first = lsel[:, 0] >= lsel[:, 1]
    i0 = np.where(first, idx2[:, 0], idx2[:, 1])
    i1 = np.where(first, idx2[:, 1], idx2[:, 0])
    l0 = np.where(first, lsel[:, 0], lsel[:, 1])
    l1 = np.where(first, lsel[:, 1], lsel[:, 0])
    e1 = np.exp((l1 - l0).astype(np.float32))
    w0 = (1.0 / (1.0 + e1)).astype(np.float32)
    w1g = (e1 / (1.0 + e1)).astype(np.float32)

    token_ids = np.concatenate([np.arange(N), np.arange(N)])
    expert_ids = np.concatenate([i0, i1])
    gate_w = np.concatenate([w0, w1g])

    counts_all = np.bincount(expert_ids, minlength=E)
    # process experts largest-first so the tail chunk is the smallest
    order = [int(e) for e in np.argsort(-counts_all) if counts_all[e] > 0]
    counts = tuple(int(counts_all[e]) for e in order)
    chunk_lists = _chunk_lists(counts)

    hb = h.astype(bf16)
    W1b = W1.astype(bf16)
    W2b = W2.astype(bf16)

    ids_per_expert = []
    gw_per_expert = []
    tok_arrays = []  # shared across cores
    for i, e in enumerate(order):
        sel = np.flatnonzero(expert_ids == e)
        ids_e = token_ids[sel]
        n_e = len(ids_e)
        ids_per_expert.append(ids_e)
        gw_per_expert.append(gate_w[sel])
        # tokens [n,D] -> [D,n] -> [KD,P,n] -> [P,KD,n]
        tokT = hb[ids_e].T.reshape(KD, P, n_e).transpose(1, 0, 2)
        c0 = 0
        arrs = []
        for cn in chunk_lists[i]:
            arrs.append(np.ascontiguousarray(tokT[:, :, c0 : c0 + cn]))
            c0 += cn
        tok_arrays.append(arrs)

    in_maps = []
    for m in range(M):
        fs = slice(m * FS, (m + 1) * FS)
        mp = {}
        for i, e in enumerate(order):
            for c, arr in enumerate(tok_arrays[i]):
                mp[f"tok{i}_{c}"] = arr
            # w1 slice [D, FS] -> [KD, P, FJ, P]
            w1s = W1b[e][:, fs].reshape(KD, P, FJ, P)
            if i == 0:
                # fused first block: [tok chunk 0 | w1 slice j=0]
                mp["first0"] = np.ascontiguousarray(
                    np.concatenate(
                        [
                            tok_arrays[0][0],
                            w1s[:, :, 0, :].transpose(1, 0, 2),
                        ],
                        axis=2,
                    )
                )
                for j in range(1, FJ):
                    mp[f"w1_0_{j}"] = np.ascontiguousarray(
                        w1s[:, :, j, :].transpose(1, 0, 2)
                    )
            else:
                mp[f"w1_{i}"] = np.ascontiguousarray(w1s.transpose(1, 2, 0, 3))
            # w2 slice [FS, D] -> [FJ, P, D] -> [P, FJ, D]
            mp[f"w2_{i}"] = np.ascontiguousarray(
                W2b[e][fs, :].reshape(FJ, P, D).transpose(1, 0, 2)
            )
        in_maps.append(mp)

    nc = _get_nc(counts)
    from concourse.bass_utils import run_bass_kernel_spmd

    LAST_RESULTS = run_bass_kernel_spmd(
        nc, in_maps, core_ids=list(range(M)), trace=TRACE
    )

    # ---- combine: sum partials over cores, then weighted scatter-add ----
    y = np.zeros((N, D), dtype=np.float32)
    for i in range(len(order)):
        ids_e = ids_per_expert[i]
        gw = gw_per_expert[i]
        col0 = 0
        for c, cn in enumerate(chunk_lists[i]):
            seg = np.zeros((P, DM, cn), dtype=np.float32)
            for m in range(M):
                seg += np.asarray(
                    LAST_RESULTS.results[m][f"out{i}_{c}"], dtype=np.float32
                )
            # [p, dm, c] -> [d, c] -> [c, d]
            seg = seg.transpose(1, 0, 2).reshape(D, cn).T
            ids_c = ids_e[col0 : col0 + cn]
            y[ids_c] += gw[col0 : col0 + cn, None] * seg
            col0 += cn
    return y.reshape(B, T, D)


# revision 12
# speedup vs baseline: 1.0007x; 1.0007x over previous
"""Trainium2 Bass kernel for nn_MoELayer (top-2 MoE, E=8 experts).

Strategy (F-sliced tensor parallel across 8 NeuronCores):
  - Host computes the (tiny) gate matmul + top-2 + softmax and groups the
    8192 token-expert pairs by expert.
  - Every core holds a distinct 512-wide slice of the F dimension of ALL 8
    experts (W1[:, :, fs], W2[:, fs, :]) and processes ALL routed pairs on
    that slice -> perfectly balanced 8192 pairs/core (no max-expert padding),
    vs expert-parallel where the busiest expert (~1070 tokens) sets the pace.
  - Each core computes partial outputs  silu(tok @ W1[e][:, fs]) @ W2[e][fs, :]
    in bf16 with fp32 PSUM accumulation; host sums the 8 partials and
    scatter-adds with the gate weights.

Startup optimizations (the baseline lost ~20us before its first matmul):
  - ~36 dummy 128-wide matmuls on a memset-zero tile issued immediately
    (no DMA dependency) warm the PE HAM clock gate (1.2 -> 2.4 GHz takes
    ~3.4us of sustained PE activity) while the first real data streams in.
  - The first real matmul only needs a 128-token micro-chunk (0.25 MB) plus
    one [P, KD, 128] W1 slice (0.25 MB) instead of 2.6 MB.
  - Weights/tokens are streamed with a 2-expert DMA lookahead; per-expert
    working set is ~4.2 MB so the queue never runs ahead of SBUF space.
  - Input loads ride the sync engine's HW DGE; per-chunk batched output
    DMAs ride the activation engine's HW DGE (separate queue).
"""

import math
import sys

sys.path.insert(0, "/opt/trn_rl_repo")

import ml_dtypes
import numpy as np

B, T, D, F, E = 2, 2048, 1024, 4096, 8
N = B * T          # 4096 tokens
NPAIRS = 2 * N     # 8192 routed token-expert pairs
P = 128
KD = D // P        # 8 contraction blocks for stage 1
DM = D // P        # 8 output-row blocks for stage 2
M = 8              # cores
FS = F // M        # 512 F-columns per core
FJ = FS // P       # 4 f-tiles per core (stage-1 output / stage-2 contraction)

NWARM = 72         # PE warm-up matmuls: bridge preamble->first data (~13us)

bf16 = ml_dtypes.bfloat16

_nc_cache: dict[tuple, object] = {}
LAST_RESULTS = None  # BassKernelResults from the most recent run (for test.py)
TRACE = False


def _chunks_for(n: int, first_small: bool, last_small: bool = False) -> list[int]:
    """Exact cover of n with chunks <=512 (PSUM bank limit), ascending.
    first_small puts a 128-token micro-chunk first so the critical first
    DMA (and first matmul) is as early as possible; last_small puts a
    128-token chunk last so the final output DMA is small."""
    if n <= 0:
        return []
    head, tail = [], []
    if first_small and n > 192:
        head.append(128)
        n -= 128
    if last_small and n > 192:
        tail.append(128)
        n -= 128
    k = math.ceil(n / 512)
    base = n // k
    rem = n - base * k
    mid = [base + (1 if i >= k - rem else 0) for i in range(k)]
    if last_small:
        mid = mid[::-1]
    return head + mid + tail


def _chunk_lists(counts) -> list[list[int]]:
    nexp = len(counts)
    return [
        _chunks_for(n, first_small=(i == 0), last_small=(i == nexp - 1))
        for i, n in enumerate(counts)
    ]


def _build(counts: tuple[int, ...]):
    import concourse.mybir as mybir
    import concourse.tile as tile
    from concourse import bacc

    dt = mybir.dt

    nc = bacc.Bacc(None, target_bir_lowering=False)

    nexp = len(counts)
    chunk_lists = _chunk_lists(counts)

    # ---- DRAM tensors ----
    tok_dram = [
        [
            nc.dram_tensor(f"tok{i}_{c}", [P, KD, cn], dt.bfloat16, kind="ExternalInput")
            for c, cn in enumerate(chunks)
        ]
        for i, chunks in enumerate(chunk_lists)
    ]
    # expert 0: first token chunk and first W1 slice fused into one tensor
    # (one DMA + one semaphore on the critical path to the first matmul),
    # remaining W1 slices fine-grained
    c00 = chunk_lists[0][0]
    first0_dram = nc.dram_tensor(
        "first0", [P, KD, c00 + P], dt.bfloat16, kind="ExternalInput"
    )
    w1_dram0 = [
        nc.dram_tensor(f"w1_0_{j}", [P, KD, P], dt.bfloat16, kind="ExternalInput")
        for j in range(1, FJ)
    ]
    w1_dram = [
        nc.dram_tensor(f"w1_{i}", [P, FJ, KD, P], dt.bfloat16, kind="ExternalInput")
        for i in range(1, nexp)
    ]
    w2_dram = [
        nc.dram_tensor(f"w2_{i}", [P, FJ, D], dt.bfloat16, kind="ExternalInput")
        for i in range(nexp)
    ]
    # partial outputs, bf16, one tensor per chunk so each DMA writes a
    # fully contiguous 5.5KB line per partition: [p, dm, c] with d = dm*128 + p
    out_dram = [
        [
            nc.dram_tensor(
                f"out{i}_{c}", [P, DM, cn], dt.bfloat16, kind="ExternalOutput"
            )
            for c, cn in enumerate(chunks)
        ]
        for i, chunks in enumerate(chunk_lists)
    ]

    with tile.TileContext(nc) as tc:
        with (
            tc.tile_pool(name="z", bufs=1) as zpool,
            tc.tile_pool(name="w", bufs=3) as wpool,
            tc.tile_pool(name="tok", bufs=10) as tokpool,
            tc.tile_pool(name="act", bufs=3) as apool,
            tc.tile_pool(name="ob", bufs=3) as opool,
            tc.tile_pool(name="ps1", bufs=4, space="PSUM") as ps1pool,
            tc.tile_pool(name="ps2", bufs=4, space="PSUM") as ps2pool,
        ):
            # ---- PE warm-up: no DMA dependency, starts right after preamble ----
            zeros = zpool.tile([P, 2 * P], dt.bfloat16, tag="zeros")
            nc.vector.memset(zeros[:], 0.0)
            for k in range(NWARM):
                wps = ps1pool.tile([P, P], dt.float32, tag="ps1")
                nc.tensor.matmul(
                    wps[:], zeros[:, 0:P], zeros[:, P : 2 * P], start=True, stop=True
                )

            tok_sb = [[None] * len(ch) for ch in chunk_lists]
            w1_sb: list = [None] * nexp
            w2_sb: list = [None] * nexp

            def emit_loads(i: int):
                chunks = chunk_lists[i]
                if i == 0:
                    # critical path: fused micro token chunk + first W1 slice
                    first0_sb = tokpool.tile(
                        [P, KD, chunks[0] + P], dt.bfloat16, tag="tok",
                        name="first0",
                    )
                    nc.sync.dma_start(first0_sb[:], first0_dram[:])
                    tok_sb[0][0] = first0_sb
                    w1_sb[0] = [first0_sb] + [
                        wpool.tile(
                            [P, KD, P], dt.bfloat16, tag=f"w1s{j}", name=f"w1s{j}"
                        )
                        for j in range(1, FJ)
                    ]
                    for j in range(1, FJ):
                        nc.sync.dma_start(w1_sb[0][j][:], w1_dram0[j - 1][:])
                    for c in range(1, len(chunks)):
                        tok_sb[0][c] = tokpool.tile(
                            [P, KD, chunks[c]], dt.bfloat16, tag="tok",
                            name=f"tok0_{c}",
                        )
                        nc.sync.dma_start(tok_sb[0][c][:], tok_dram[0][c][:])
                else:
                    for c, cn in enumerate(chunks):
                        tok_sb[i][c] = tokpool.tile(
                            [P, KD, cn], dt.bfloat16, tag="tok",
                            name=f"tok{i}_{c}",
                        )
                        nc.sync.dma_start(tok_sb[i][c][:], tok_dram[i][c][:])
                    w1_sb[i] = wpool.tile(
                        [P, FJ, KD, P], dt.bfloat16, tag="w1", name=f"w1_{i}"
                    )
                    nc.sync.dma_start(w1_sb[i][:], w1_dram[i - 1][:])
                w2_sb[i] = wpool.tile(
                    [P, FJ, D], dt.bfloat16, tag="w2", name=f"w2_{i}"
                )
                nc.sync.dma_start(w2_sb[i][:], w2_dram[i][:])

            emit_loads(0)
            if nexp > 1:
                emit_loads(1)

            outno = 0
            for i in range(nexp):
                if i + 2 < nexp:
                    emit_loads(i + 2)
                chunks = chunk_lists[i]
                # ---- stage 1: actT[f_local, c] = silu(W1s^T @ tokT) ----
                act_sbs = []
                for c, cn in enumerate(chunks):
                    act_sb = apool.tile([P, FJ, cn], dt.bfloat16, tag="act")
                    act_sbs.append(act_sb)
                    c0w = chunk_lists[0][0]
                    for j in range(FJ):
                        ps1 = ps1pool.tile([P, cn], dt.float32, tag="ps1")
                        for dk in range(KD):
                            if i == 0 and j == 0:
                                lhs = w1_sb[0][0][:, dk, c0w : c0w + P]
                            elif i == 0:
                                lhs = w1_sb[0][j][:, dk, :]
                            else:
                                lhs = w1_sb[i][:, j, dk, :]
                            nc.tensor.matmul(
                                ps1[:],
                                lhs,
                                tok_sb[i][c][:, dk, : chunks[0]]
                                if (i == 0 and c == 0)
                                else tok_sb[i][c][:, dk, :],
                                start=(dk == 0),
                                stop=(dk == KD - 1),
                            )
                        nc.scalar.activation(
                            act_sb[:, j, :],
                            ps1[:],
                            mybir.ActivationFunctionType.Silu,
                        )
                # ---- stage 2: outT[d, c] = W2s^T @ actT (partial over f) ----
                for c, cn in enumerate(chunks):
                    ob = opool.tile([P, DM, cn], dt.bfloat16, tag="ob")
                    for dm in range(DM):
                        ps2 = ps2pool.tile([P, cn], dt.float32, tag="ps2")
                        for fk in range(FJ):
                            nc.tensor.matmul(
                                ps2[:],
                                w2_sb[i][:, fk, dm * P : (dm + 1) * P],
                                act_sbs[c][:, fk, :],
                                start=(fk == 0),
                                stop=(fk == FJ - 1),
                            )
                        nc.vector.tensor_copy(ob[:, dm, :], ps2[:])
                    if i == nexp - 1 and c >= len(chunks) - 2:
                        # tail chunks: split the DMA across both HWDGE queues
                        # so the post-compute drain is halved
                        nc.scalar.dma_start(
                            out_dram[i][c][:, : DM // 2, :], ob[:, : DM // 2, :]
                        )
                        nc.sync.dma_start(
                            out_dram[i][c][:, DM // 2 :, :], ob[:, DM // 2 :, :]
                        )
                    else:
                        # alternate output DMAs across the two HWDGE queues
                        eng = nc.scalar if outno % 2 == 0 else nc.sync
                        eng.dma_start(out_dram[i][c][:], ob[:])
                    outno += 1

    nc.compile()
    return nc


def _get_nc(counts: tuple[int, ...]):
    if counts not in _nc_cache:
        _nc_cache[counts] = _build(counts)
    return _nc_cache[counts]


def kernel(**inputs) -> np.ndarray:
    global LAST_RESULTS
    x = np.asarray(inputs["x"], dtype=np.float32)
    Wg = np.asarray(inputs["Wg"], dtype=np.float32)
    W1 = np.asarray(inputs["W1"], dtype=np.float32)
    W2 = np.asarray(inputs["W2"], dtype=np.float32)

    h = np.ascontiguousarray(x.reshape(N, D))

    # ---- host gate: top-2 + softmax (0.05% of total FLOPs) ----
    logits = h @ Wg.T  # [N, E] f32
    idx2 = np.argpartition(-logits, 1, axis=1)[:, :2]
    lsel = np.take_along_axis(logits, idx2, axis=1)
    first = lsel[:, 0] >= lsel[:, 1]
    i0 = np.where(first, idx2[:, 0], idx2[:, 1])
    i1 = np.where(first, idx2[:, 1], idx2[:, 0])
    l0 = np.where(first, lsel[:, 0], lsel[:, 1])
    l1 = np.where(first, lsel[:, 1], lsel[:, 0])
    e1 = np.exp((l1 - l0).astype(np.float32))
    w0 = (1.0 / (1.0 + e1)).astype(np.float32)
    w1g = (e1 / (1.0 + e1)).astype(np.float32)

    token_ids = np.concatenate([np.arange(N), np.arange(N)])
    expert_ids = np.concatenate([i0, i1])
    gate_w = np.concatenate([w0, w1g])

    counts_all = np.bincount(expert_ids, minlength=E)
    # process experts largest-first so the tail chunk is the smallest
    order = [int(e) for e in np.argsort(-counts_all) if counts_all[e] > 0]
    counts = tuple(int(counts_all[e]) for e in order)
    chunk_lists = _chunk_lists(counts)

    hb = h.astype(bf16)
    W1b = W1.astype(bf16)
    W2b = W2.astype(bf16)

    ids_per_expert = []
    gw_per_expert = []
    tok_arrays = []  # shared across cores
    for i, e in enumerate(order):
        sel = np.flatnonzero(expert_ids == e)
        ids_e = token_ids[sel]
        n_e = len(ids_e)
        ids_per_expert.append(ids_e)
        gw_per_expert.append(gate_w[sel])
        # tokens [n,D] -> [D,n] -> [KD,P,n] -> [P,KD,n]
        tokT = hb[ids_e].T.reshape(KD, P, n_e).transpose(1, 0, 2)
        c0 = 0
        arrs = []
        for cn in chunk_lists[i]:
            arrs.append(np.ascontiguousarray(tokT[:, :, c0 : c0 + cn]))
            c0 += cn
        tok_arrays.append(arrs)

    in_maps = []
    for m in range(M):
        fs = slice(m * FS, (m + 1) * FS)
        mp = {}
        for i, e in enumerate(order):
            for c, arr in enumerate(tok_arrays[i]):
                mp[f"tok{i}_{c}"] = arr
            # w1 slice [D, FS] -> [KD, P, FJ, P]
            w1s = W1b[e][:, fs].reshape(KD, P, FJ, P)
            if i == 0:
                # fused first block: [tok chunk 0 | w1 slice j=0]
                mp["first0"] = np.ascontiguousarray(
                    np.concatenate(
                        [
                            tok_arrays[0][0],
                            w1s[:, :, 0, :].transpose(1, 0, 2),
                        ],
                        axis=2,
                    )
                )
                for j in range(1, FJ):
                    mp[f"w1_0_{j}"] = np.ascontiguousarray(
                        w1s[:, :, j, :].transpose(1, 0, 2)
                    )
            else:
                mp[f"w1_{i}"] = np.ascontiguousarray(w1s.transpose(1, 2, 0, 3))
            # w2 slice [FS, D] -> [FJ, P, D] -> [P, FJ, D]
            mp[f"w2_{i}"] = np.ascontiguousarray(
                W2b[e][fs, :].reshape(FJ, P, D).transpose(1, 0, 2)
            )
        in_maps.append(mp)

    nc = _get_nc(counts)
    from concourse.bass_utils import run_bass_kernel_spmd

    LAST_RESULTS = run_bass_kernel_spmd(
        nc, in_maps, core_ids=list(range(M)), trace=TRACE
    )

    # ---- combine: sum partials over cores, then weighted scatter-add ----
    y = np.zeros((N, D), dtype=np.float32)
    for i in range(len(order)):
        ids_e = ids_per_expert[i]
        gw = gw_per_expert[i]
        col0 = 0
        for c, cn in enumerate(chunk_lists[i]):
            seg = np.zeros((P, DM, cn), dtype=np.float32)
            for m in range(M):
                seg += np.asarray(
                    LAST_RESULTS.results[m][f"out{i}_{c}"], dtype=np.float32
                )
            # [p, dm, c] -> [d, c] -> [c, d]
            seg = seg.transpose(1, 0, 2).reshape(D, cn).T
            ids_c = ids_e[col0 : col0 + cn]
            y[ids_c] += gw[col0 : col0 + cn, None] * seg
            col0 += cn
    return y.reshape(B, T, D)


# revision 13
# speedup vs baseline: 1.0021x; 1.0014x over previous
"""Trainium2 Bass kernel for nn_MoELayer (top-2 MoE, E=8 experts).

Strategy (F-sliced tensor parallel across 8 NeuronCores):
  - Host computes the (tiny) gate matmul + top-2 + softmax and groups the
    8192 token-expert pairs by expert.
  - Every core holds a distinct 512-wide slice of the F dimension of ALL 8
    experts (W1[:, :, fs], W2[:, fs, :]) and processes ALL routed pairs on
    that slice -> perfectly balanced 8192 pairs/core (no max-expert padding),
    vs expert-parallel where the busiest expert (~1070 tokens) sets the pace.
  - Each core computes partial outputs  silu(tok @ W1[e][:, fs]) @ W2[e][fs, :]
    in bf16 with fp32 PSUM accumulation; host sums the 8 partials and
    scatter-adds with the gate weights.

Startup/tail optimizations (the baseline lost ~20us before its first
matmul and ~6us after its last):
  - NWARM dummy 128-wide matmuls on a memset-zero tile issued immediately
    (no DMA dependency) warm the PE HAM clock gate (1.2 -> 2.4 GHz needs
    ~3.4us of sustained PE activity) and keep the PE busy until the first
    real data lands (~12.5us: NEFF preamble + 0.5 MB critical DMA). Any
    PE idle >~1us here lets HAM re-throttle to half clock for several us.
  - The critical first DMA is a single fused tensor (first token
    micro-chunk + first W1 slice, 0.5 MB) -> one transfer, one semaphore.
  - Weights/tokens are streamed with a 2-expert DMA lookahead; per-expert
    working set is ~4.2 MB so SBUF never overflows and HBM stays ahead.
  - Input loads ride the sync engine's HW DGE; per-chunk batched output
    DMAs (contiguous 5.5KB lines; 256B lines would clog the DMA engines)
    alternate between the two HW DGE queues (sync + activation), and the
    final two chunks split each DMA across both queues to halve the
    post-compute drain. The last expert processed is the smallest and
    ends with a 128-token chunk so the final output DMA is tiny.
"""

import math
import sys

sys.path.insert(0, "/opt/trn_rl_repo")

import ml_dtypes
import numpy as np

B, T, D, F, E = 2, 2048, 1024, 4096, 8
N = B * T          # 4096 tokens
NPAIRS = 2 * N     # 8192 routed token-expert pairs
P = 128
KD = D // P        # 8 contraction blocks for stage 1
DM = D // P        # 8 output-row blocks for stage 2
M = 8              # cores
FS = F // M        # 512 F-columns per core
FJ = FS // P       # 4 f-tiles per core (stage-1 output / stage-2 contraction)

NWARM = 72         # PE warm-up matmuls: bridge preamble->first data (~13us)

bf16 = ml_dtypes.bfloat16

_nc_cache: dict[tuple, object] = {}
LAST_RESULTS = None  # BassKernelResults from the most recent run (for test.py)
TRACE = False


def _chunks_for(n: int, first_small: bool, last_small: bool = False) -> list[int]:
    """Exact cover of n with chunks <=512 (PSUM bank limit), ascending.
    first_small puts a 128-token micro-chunk first so the critical first
    DMA (and first matmul) is as early as possible; last_small puts a
    128-token chunk last so the final output DMA is small."""
    if n <= 0:
        return []
    head, tail = [], []
    if first_small and n > 192:
        head.append(128)
        n -= 128
    if last_small and n > 192:
        tail.append(128)
        n -= 128
    k = math.ceil(n / 512)
    base = n // k
    rem = n - base * k
    mid = [base + (1 if i >= k - rem else 0) for i in range(k)]
    if last_small:
        mid = mid[::-1]
    return head + mid + tail


def _chunk_lists(counts) -> list[list[int]]:
    nexp = len(counts)
    return [
        _chunks_for(n, first_small=(i == 0), last_small=(i == nexp - 1))
        for i, n in enumerate(counts)
    ]


def _build(counts: tuple[int, ...]):
    import concourse.mybir as mybir
    import concourse.tile as tile
    from concourse import bacc

    dt = mybir.dt

    nc = bacc.Bacc(None, target_bir_lowering=False)

    nexp = len(counts)
    chunk_lists = _chunk_lists(counts)

    # ---- DRAM tensors ----
    tok_dram = [
        [
            nc.dram_tensor(f"tok{i}_{c}", [P, KD, cn], dt.bfloat16, kind="ExternalInput")
            for c, cn in enumerate(chunks)
        ]
        for i, chunks in enumerate(chunk_lists)
    ]
    # expert 0: first token chunk and first W1 slice fused into one tensor
    # (one DMA + one semaphore on the critical path to the first matmul),
    # remaining W1 slices fine-grained
    c00 = chunk_lists[0][0]
    first0_dram = nc.dram_tensor(
        "first0", [P, KD, c00 + P], dt.bfloat16, kind="ExternalInput"
    )
    w1_dram0 = [
        nc.dram_tensor(f"w1_0_{j}", [P, KD, P], dt.bfloat16, kind="ExternalInput")
        for j in range(1, FJ)
    ]
    w1_dram = [
        nc.dram_tensor(f"w1_{i}", [P, FJ, KD, P], dt.bfloat16, kind="ExternalInput")
        for i in range(1, nexp)
    ]
    w2_dram = [
        nc.dram_tensor(f"w2_{i}", [P, FJ, D], dt.bfloat16, kind="ExternalInput")
        for i in range(nexp)
    ]
    # partial outputs, bf16, one tensor per chunk so each DMA writes a
    # fully contiguous 5.5KB line per partition: [p, dm, c] with d = dm*128 + p
    out_dram = [
        [
            nc.dram_tensor(
                f"out{i}_{c}", [P, DM, cn], dt.bfloat16, kind="ExternalOutput"
            )
            for c, cn in enumerate(chunks)
        ]
        for i, chunks in enumerate(chunk_lists)
    ]

    with tile.TileContext(nc) as tc:
        with (
            tc.tile_pool(name="z", bufs=1) as zpool,
            tc.tile_pool(name="w", bufs=3) as wpool,
            tc.tile_pool(name="tok", bufs=10) as tokpool,
            tc.tile_pool(name="act", bufs=3) as apool,
            tc.tile_pool(name="ob", bufs=3) as opool,
            tc.tile_pool(name="ps1", bufs=4, space="PSUM") as ps1pool,
            tc.tile_pool(name="ps2", bufs=4, space="PSUM") as ps2pool,
        ):
            # ---- PE warm-up: no DMA dependency, starts right after preamble ----
            zeros = zpool.tile([P, 2 * P], dt.bfloat16, tag="zeros")
            nc.vector.memset(zeros[:], 0.0)
            for k in range(NWARM):
                wps = ps1pool.tile([P, P], dt.float32, tag="ps1")
                nc.tensor.matmul(
                    wps[:], zeros[:, 0:P], zeros[:, P : 2 * P], start=True, stop=True
                )

            tok_sb = [[None] * len(ch) for ch in chunk_lists]
            w1_sb: list = [None] * nexp
            w2_sb: list = [None] * nexp

            def emit_loads(i: int):
                chunks = chunk_lists[i]
                if i == 0:
                    # critical path: fused micro token chunk + first W1 slice
                    first0_sb = tokpool.tile(
                        [P, KD, chunks[0] + P], dt.bfloat16, tag="tok",
                        name="first0",
                    )
                    nc.sync.dma_start(first0_sb[:], first0_dram[:])
                    tok_sb[0][0] = first0_sb
                    w1_sb[0] = [first0_sb] + [
                        wpool.tile(
                            [P, KD, P], dt.bfloat16, tag=f"w1s{j}", name=f"w1s{j}"
                        )
                        for j in range(1, FJ)
                    ]
                    for j in range(1, FJ):
                        nc.sync.dma_start(w1_sb[0][j][:], w1_dram0[j - 1][:])
                    for c in range(1, len(chunks)):
                        tok_sb[0][c] = tokpool.tile(
                            [P, KD, chunks[c]], dt.bfloat16, tag="tok",
                            name=f"tok0_{c}",
                        )
                        nc.sync.dma_start(tok_sb[0][c][:], tok_dram[0][c][:])
                else:
                    for c, cn in enumerate(chunks):
                        tok_sb[i][c] = tokpool.tile(
                            [P, KD, cn], dt.bfloat16, tag="tok",
                            name=f"tok{i}_{c}",
                        )
                        nc.sync.dma_start(tok_sb[i][c][:], tok_dram[i][c][:])
                    w1_sb[i] = wpool.tile(
                        [P, FJ, KD, P], dt.bfloat16, tag="w1", name=f"w1_{i}"
                    )
                    nc.sync.dma_start(w1_sb[i][:], w1_dram[i - 1][:])
                w2_sb[i] = wpool.tile(
                    [P, FJ, D], dt.bfloat16, tag="w2", name=f"w2_{i}"
                )
                nc.sync.dma_start(w2_sb[i][:], w2_dram[i][:])

            emit_loads(0)
            if nexp > 1:
                emit_loads(1)

            outno = 0
            for i in range(nexp):
                if i + 2 < nexp:
                    emit_loads(i + 2)
                chunks = chunk_lists[i]
                # ---- stage 1: actT[f_local, c] = silu(W1s^T @ tokT) ----
                act_sbs = []
                for c, cn in enumerate(chunks):
                    act_sb = apool.tile([P, FJ, cn], dt.bfloat16, tag="act")
                    act_sbs.append(act_sb)
                    c0w = chunk_lists[0][0]
                    for j in range(FJ):
                        ps1 = ps1pool.tile([P, cn], dt.float32, tag="ps1")
                        for dk in range(KD):
                            if i == 0 and j == 0:
                                lhs = w1_sb[0][0][:, dk, c0w : c0w + P]
                            elif i == 0:
                                lhs = w1_sb[0][j][:, dk, :]
                            else:
                                lhs = w1_sb[i][:, j, dk, :]
                            nc.tensor.matmul(
                                ps1[:],
                                lhs,
                                tok_sb[i][c][:, dk, : chunks[0]]
                                if (i == 0 and c == 0)
                                else tok_sb[i][c][:, dk, :],
                                start=(dk == 0),
                                stop=(dk == KD - 1),
                            )
                        nc.scalar.activation(
                            act_sb[:, j, :],
                            ps1[:],
                            mybir.ActivationFunctionType.Silu,
                        )
                # ---- stage 2: outT[d, c] = W2s^T @ actT (partial over f) ----
                for c, cn in enumerate(chunks):
                    ob = opool.tile([P, DM, cn], dt.bfloat16, tag="ob")
                    for dm in range(DM):
                        ps2 = ps2pool.tile([P, cn], dt.float32, tag="ps2")
                        for fk in range(FJ):
                            nc.tensor.matmul(
                                ps2[:],
                                w2_sb[i][:, fk, dm * P : (dm + 1) * P],
                                act_sbs[c][:, fk, :],
                                start=(fk == 0),
                                stop=(fk == FJ - 1),
                            )
                        nc.vector.tensor_copy(ob[:, dm, :], ps2[:])
                    if i == nexp - 1 and c >= len(chunks) - 2:
                        # tail chunks: split the DMA across both HWDGE queues
                        # so the post-compute drain is halved
                        nc.scalar.dma_start(
                            out_dram[i][c][:, : DM // 2, :], ob[:, : DM // 2, :]
                        )
                        nc.sync.dma_start(
                            out_dram[i][c][:, DM // 2 :, :], ob[:, DM // 2 :, :]
                        )
                    else:
                        # alternate output DMAs across the two HWDGE queues
                        eng = nc.scalar if outno % 2 == 0 else nc.sync
                        eng.dma_start(out_dram[i][c][:], ob[:])
                    outno += 1

    nc.compile()
    return nc


def _get_nc(counts: tuple[int, ...]):
    if counts not in _nc_cache:
        _nc_cache[counts] = _build(counts)
    return _nc_cache[counts]


def kernel(**inputs) -> np.ndarray:
    global LAST_RESULTS
    x = np.asarray(inputs["x"], dtype=np.float32)
    Wg = np.asarray(inputs["Wg"], dtype=np.float32)
    W1 = np.asarray(inputs["W1"], dtype=np.float32)
    W2 = np.asarray(inputs["W2"], dtype=np.float32)

    h = np.ascontiguousarray(x.reshape(N, D))

    # ---- host gate: top-2 + softmax (0.05% of total FLOPs) ----
    logits = h @ Wg.T  # [N, E] f32
    idx2 = np.argpartition(-logits, 1, axis=1)[:, :2]
    lsel = np.take_along_axis(logits, idx2, axis=1)
    first = lsel[:, 0] >= lsel[:, 1]
    i0 = np.where(first, idx2[:, 0], idx2[:, 1])
    i1 = np.where(first, idx2[:, 1], idx2[:, 0])
    l0 = np.where(first, lsel[:, 0], lsel[:, 1])
    l1 = np.where(first, lsel[:, 1], lsel[:, 0])
    e1 = np.exp((l1 - l0).astype(np.float32))
    w0 = (1.0 / (1.0 + e1)).astype(np.float32)
    w1g = (e1 / (1.0 + e1)).astype(np.float32)

    token_ids = np.concatenate([np.arange(N), np.arange(N)])
    expert_ids = np.concatenate([i0, i1])
    gate_w = np.concatenate([w0, w1g])

    counts_all = np.bincount(expert_ids, minlength=E)
    # process experts largest-first so the tail chunk is the smallest
    order = [int(e) for e in np.argsort(-counts_all) if counts_all[e] > 0]
    counts = tuple(int(counts_all[e]) for e in order)
    chunk_lists = _chunk_lists(counts)

    hb = h.astype(bf16)
    W1b = W1.astype(bf16)
    W2b = W2.astype(bf16)

    ids_per_expert = []
    gw_per_expert = []
    tok_arrays = []  # shared across cores
    for i, e in enumerate(order):
        sel = np.flatnonzero(expert_ids == e)
        ids_e = token_ids[sel]
        n_e = len(ids_e)
        ids_per_expert.append(ids_e)
        gw_per_expert.append(gate_w[sel])
        # tokens [n,D] -> [D,n] -> [KD,P,n] -> [P,KD,n]
        tokT = hb[ids_e].T.reshape(KD, P, n_e).transpose(1, 0, 2)
        c0 = 0
        arrs = []
        for cn in chunk_lists[i]:
            arrs.append(np.ascontiguousarray(tokT[:, :, c0 : c0 + cn]))
            c0 += cn
        tok_arrays.append(arrs)

    in_maps = []
    for m in range(M):
        fs = slice(m * FS, (m + 1) * FS)
        mp = {}
        for i, e in enumerate(order):
            for c, arr in enumerate(tok_arrays[i]):
                mp[f"tok{i}_{c}"] = arr
            # w1 slice [D, FS] -> [KD, P, FJ, P]
            w1s = W1b[e][:, fs].reshape(KD, P, FJ, P)
            if i == 0:
                # fused first block: [tok chunk 0 | w1 slice j=0]
                mp["first0"] = np.ascontiguousarray(
                    np.concatenate(
                        [
                            tok_arrays[0][0],
                            w1s[:, :, 0, :].transpose(1, 0, 2),
                        ],
                        axis=2,
                    )
                )
                for j in range(1, FJ):
                    mp[f"w1_0_{j}"] = np.ascontiguousarray(
                        w1s[:, :, j, :].transpose(1, 0, 2)
                    )
            else:
                mp[f"w1_{i}"] = np.ascontiguousarray(w1s.transpose(1, 2, 0, 3))
            # w2 slice [FS, D] -> [FJ, P, D] -> [P, FJ, D]
            mp[f"w2_{i}"] = np.ascontiguousarray(
                W2b[e][fs, :].reshape(FJ, P, D).transpose(1, 0, 2)
            )
        in_maps.append(mp)

    nc = _get_nc(counts)
    from concourse.bass_utils import run_bass_kernel_spmd

    LAST_RESULTS = run_bass_kernel_spmd(
        nc, in_maps, core_ids=list(range(M)), trace=TRACE
    )

    # ---- combine: sum partials over cores, then weighted scatter-add ----
    y = np.zeros((N, D), dtype=np.float32)
    for i in range(len(order)):
        ids_e = ids_per_expert[i]
        gw = gw_per_expert[i]
        col0 = 0
        for c, cn in enumerate(chunk_lists[i]):
            seg = np.zeros((P, DM, cn), dtype=np.float32)
            for m in range(M):
                seg += np.asarray(
                    LAST_RESULTS.results[m][f"out{i}_{c}"], dtype=np.float32
                )
            # [p, dm, c] -> [d, c] -> [c, d]
            seg = seg.transpose(1, 0, 2).reshape(D, cn).T
            ids_c = ids_e[col0 : col0 + cn]
            y[ids_c] += gw[col0 : col0 + cn, None] * seg
            col0 += cn
    return y.reshape(B, T, D)
